# revision 5
# baseline (speedup 1.0000x reference)
"""Sliding-window GQA causal self-attention block for 8 trn2 NeuronCores.

Sharding: batch (4) x T-halves (2) -> 8 cores, no collectives. Each core gets
x.T for its T-half plus a 256-row key/value halo and computes its (1024, 1024)
slice of the output.

Scores are computed keys-on-partitions (S.T) per (kv-group, q-block); exp on
ACT with scale=1/8 (softmax without max-subtraction -- scores are O(5));
band masks via gpsimd affine_select on the exp'd tile; att@v uses a ones
column appended to v so the softmax denominator falls out of the same matmul;
normalization via reciprocal + partition-broadcast DMA.
"""

import dataclasses

import numpy as np
import ml_dtypes

import concourse.bass as bass
import concourse.mybir as mybir
import concourse.tile as tile
from concourse import bacc
from concourse.bass_utils import run_bass_kernel_spmd

BF = ml_dtypes.bfloat16
F32 = mybir.dt.float32
BF16 = mybir.dt.bfloat16

B, T, C = 4, 2048, 1024
H, KV, HD = 16, 4, 64
WIN = 256
TL = T // 2            # 1024 own rows per core
TH = TL + WIN          # 1280 with halo
NEG = -30000.0


def _build_program():
    nc = bacc.Bacc("TRN2", target_bir_lowering=False, debug=False, num_devices=8)
    dt = mybir.dt
    xT = nc.dram_tensor("xT", [C, TH], dt.bfloat16, kind="ExternalInput").ap()
    wqk = nc.dram_tensor("wqk", [C, 1280], dt.bfloat16, kind="ExternalInput").ap()
    wv = nc.dram_tensor("wv", [C, 256], dt.bfloat16, kind="ExternalInput").ap()
    wp = nc.dram_tensor("wp", [C, C], dt.bfloat16, kind="ExternalInput").ap()
    cq = nc.dram_tensor("cq", [2, 128, TL], dt.bfloat16, kind="ExternalInput").ap()
    ck = nc.dram_tensor("ck", [2, 128, TH], dt.bfloat16, kind="ExternalInput").ap()
    vb = nc.dram_tensor("vb", [1, 640], dt.float32, kind="ExternalInput").ap()
    out = nc.dram_tensor("out", [TL, C], dt.float32, kind="ExternalOutput").ap()

    with tile.TileContext(nc) as tc:
        _kernel_body(tc, nc, xT, wqk, wv, wp, cq, ck, vb, out)
    nc.compile()
    return nc


def _kernel_body(tc, nc, xT, wqk, wv, wp, cq, ck, vb, out):
    import contextlib
    ctx = contextlib.ExitStack()
    with ctx:
        consts = ctx.enter_context(tc.tile_pool(name="consts", bufs=1))
        persist = ctx.enter_context(tc.tile_pool(name="persist", bufs=1))

        # ---- load persistent inputs ----
        xT_sb, wqk_sb, wv_sb, wp_sb = [], [], [], []
        for kc in range(8):
            t = persist.tile([128, TH], BF16, tag=f"xT{kc}")
            nc.gpsimd.dma_start(out=t[:], in_=xT[kc * 128:(kc + 1) * 128, :])
            xT_sb.append(t)
            t = persist.tile([128, 1280], BF16, tag=f"wqk{kc}")
            nc.gpsimd.dma_start(out=t[:], in_=wqk[kc * 128:(kc + 1) * 128, :])
            wqk_sb.append(t)
            t = persist.tile([128, 256], BF16, tag=f"wv{kc}")
            nc.gpsimd.dma_start(out=t[:], in_=wv[kc * 128:(kc + 1) * 128, :])
            wv_sb.append(t)
            t = persist.tile([128, C], BF16, tag=f"wp{kc}")
            nc.gpsimd.dma_start(out=t[:], in_=wp[kc * 128:(kc + 1) * 128, :])
            wp_sb.append(t)
        cq_sb = consts.tile([128, 2, TL], BF16)
        nc.gpsimd.dma_start(out=cq_sb[:, 0, :], in_=cq[0])
        nc.gpsimd.dma_start(out=cq_sb[:, 1, :], in_=cq[1])
        ck_sb = consts.tile([128, 2, TH], BF16)
        nc.gpsimd.dma_start(out=ck_sb[:, 0, :], in_=ck[0])
        nc.gpsimd.dma_start(out=ck_sb[:, 1, :], in_=ck[1])
        vb_sb = consts.tile([1, 640], F32)
        nc.gpsimd.dma_start(out=vb_sb[:], in_=vb)
        ones_sb = consts.tile([1, 512], F32)
        nc.vector.memset(ones_sb[:], 1.0)

        # persistent compute tensors
        qT = [persist.tile([64, TL], BF16, tag=f"qT{h}", name=f"qT{h}") for h in range(H)]
        kT = [persist.tile([64, TH], BF16, tag=f"kT{g}", name=f"kT{g}") for g in range(KV)]
        v65 = [persist.tile([128, 4 * 65], BF16, tag=f"v65_{i}", name=f"v65_{i}") for i in range(10)]
        yTn = persist.tile([128, 8 * TL], BF16, tag="yTn")  # paired heads x T

        # ======== phase 1: qkv projection + rope ========
        with tc.tile_pool(name="pps", bufs=1, space="PSUM") as pps, \
             tc.tile_pool(name="vps", bufs=2, space="PSUM") as vps, \
             tc.tile_pool(name="ropes", bufs=2) as ropes:

            def rope_pair(pe, po, cs_sb, tlen):
                e_sb = ropes.tile([128, tlen], BF16, tag="e_sb")
                o_sb = ropes.tile([128, tlen], BF16, tag="o_sb")
                nc.vector.tensor_copy(e_sb[:], pe[:, 0:tlen])
                nc.vector.tensor_copy(o_sb[:], po[:, 0:tlen])
                ne = ropes.tile([128, tlen], BF16, tag="r0")
                no_ = ropes.tile([128, tlen], BF16, tag="r1")
                t1 = ropes.tile([128, tlen], BF16, tag="r2")
                t2 = ropes.tile([128, tlen], BF16, tag="r3")
                nc.vector.tensor_mul(t1[:], e_sb[:], cs_sb[:, 0, 0:tlen])
                nc.vector.tensor_mul(t2[:], o_sb[:], cs_sb[:, 1, 0:tlen])
                nc.vector.tensor_sub(ne[:], t1[:], t2[:])
                nc.vector.tensor_mul(t1[:], e_sb[:], cs_sb[:, 1, 0:tlen])
                nc.vector.tensor_mul(t2[:], o_sb[:], cs_sb[:, 0, 0:tlen])
                nc.vector.tensor_add(no_[:], t1[:], t2[:])
                return ne, no_

            # q: wqk cols [0:512]=all-heads-evens, [512:1024]=all-heads-odds
            for c4 in range(4):
                pe = pps.tile([128, TH], F32, tag="pe")
                po = pps.tile([128, TH], F32, tag="po")
                for half in range(2):
                    for kc in range(8):
                        nc.tensor.matmul(
                            pe[:, half * 512:(half + 1) * 512],
                            wqk_sb[kc][:, c4 * 128:(c4 + 1) * 128],
                            xT_sb[kc][:, WIN + half * 512:WIN + (half + 1) * 512],
                            start=(kc == 0), stop=(kc == 7))
                    for kc in range(8):
                        nc.tensor.matmul(
                            po[:, half * 512:(half + 1) * 512],
                            wqk_sb[kc][:, 512 + c4 * 128:512 + (c4 + 1) * 128],
                            xT_sb[kc][:, WIN + half * 512:WIN + (half + 1) * 512],
                            start=(kc == 0), stop=(kc == 7))
                ne, no_ = rope_pair(pe, po, cq_sb, TL)
                for j in range(4):
                    h = c4 * 4 + j
                    nc.gpsimd.dma_start(out=qT[h][0:32, :],
                                        in_=ne[j * 32:(j + 1) * 32, :])
                    nc.gpsimd.dma_start(out=qT[h][32:64, :],
                                        in_=no_[j * 32:(j + 1) * 32, :])

            # k: wqk cols [1024:1152]=kv evens, [1152:1280]=kv odds, full TH
            pe = pps.tile([128, TH], F32, tag="pe")
            po = pps.tile([128, TH], F32, tag="po")
            for (n0, n1) in ((0, 512), (512, 1024), (1024, 1280)):
                for kc in range(8):
                    nc.tensor.matmul(pe[:, n0:n1], wqk_sb[kc][:, 1024:1152],
                                     xT_sb[kc][:, n0:n1],
                                     start=(kc == 0), stop=(kc == 7))
                for kc in range(8):
                    nc.tensor.matmul(po[:, n0:n1], wqk_sb[kc][:, 1152:1280],
                                     xT_sb[kc][:, n0:n1],
                                     start=(kc == 0), stop=(kc == 7))
            ne, no_ = rope_pair(pe, po, ck_sb, TH)
            for g in range(KV):
                nc.gpsimd.dma_start(out=kT[g][0:32, :],
                                    in_=ne[g * 32:(g + 1) * 32, :])
                nc.gpsimd.dma_start(out=kT[g][32:64, :],
                                    in_=no_[g * 32:(g + 1) * 32, :])

            # v: natural layout (t partitions, 4 heads x 64) + ones column
            for tcn in range(10):
                pv = vps.tile([128, 256], F32, tag="pv")
                for kc in range(8):
                    nc.tensor.matmul(pv[:], xT_sb[kc][:, tcn * 128:(tcn + 1) * 128],
                                     wv_sb[kc][:], start=(kc == 0), stop=(kc == 7))
                v3 = v65[tcn][:].rearrange("p (g c) -> p g c", c=65)
                nc.vector.tensor_copy(v3[:, :, 0:64],
                                      pv[:].rearrange("p (g c) -> p g c", c=64))
                nc.vector.memset(v3[:, :, 64:65], 1.0)

        # ======== phase 2: attention ========
        with tc.tile_pool(name="stps", bufs=2, space="PSUM") as stps, \
             tc.tile_pool(name="yups", bufs=2, space="PSUM") as yups, \
             tc.tile_pool(name="atts", bufs=3) as atts:
            for qb in range(8):
                for g in range(KV):
                    st = stps.tile([128, 4 * 384], F32, tag="st")
                    for j in range(4):          # 4 q-heads of this kv group
                        h = 4 * g + j
                        base = j * 384
                        for cc in range(3):     # key chunks qb, qb+1, qb+2
                            has_vb = (qb + cc) <= 1
                            nc.tensor.matmul(
                                st[:, base + cc * 128:base + (cc + 1) * 128],
                                kT[g][:, (qb + cc) * 128:(qb + cc + 1) * 128],
                                qT[h][:, qb * 128:(qb + 1) * 128],
                                start=True, stop=not has_vb)
                            if has_vb:
                                nc.tensor.matmul(
                                    st[:, base + cc * 128:base + (cc + 1) * 128],
                                    vb_sb[:, (qb + cc) * 128:(qb + cc + 1) * 128],
                                    ones_sb[:, 0:128],
                                    start=False, stop=True)
                    pt = atts.tile([128, 4 * 384], BF16, tag="pt")
                    nc.scalar.activation(pt[:], st[:],
                                         mybir.ActivationFunctionType.Exp,
                                         scale=0.125)
                    pt3 = pt[:].rearrange("p (h k) -> p h k", k=384)
                    # band mask chunk0: keep iff k_rel >= q_rel+1  (p - f - 1 >= 0)
                    nc.gpsimd.affine_select(
                        out=pt3[:, :, 0:128], in_=pt3[:, :, 0:128],
                        compare_op=mybir.AluOpType.is_ge, fill=0.0,
                        base=-1, channel_multiplier=1, pattern=[[0, 4], [-1, 128]])
                    # band mask chunk2: keep iff k_rel <= q_rel+256  (f - p >= 0)
                    nc.gpsimd.affine_select(
                        out=pt3[:, :, 256:384], in_=pt3[:, :, 256:384],
                        compare_op=mybir.AluOpType.is_ge, fill=0.0,
                        base=0, channel_multiplier=-1, pattern=[[0, 4], [1, 128]])
                    yu = yups.tile([65, 512], F32, tag="yu")
                    for j in range(4):
                        for cc in range(3):
                            nc.tensor.matmul(
                                yu[:, j * 128:(j + 1) * 128],
                                v65[qb + cc][:, g * 65:(g + 1) * 65],
                                pt3[:, j, cc * 128:(cc + 1) * 128],
                                start=(cc == 0), stop=(cc == 2))
                    # denominators -> reciprocal -> broadcast -> normalize
                    r_sb = atts.tile([1, 512], F32, tag="r_sb")
                    nc.vector.reciprocal(r_sb[:], yu[64:65, :])
                    bc_e = atts.tile([64, 2, 128], F32, tag="bc_e")
                    bc_o = atts.tile([64, 2, 128], F32, tag="bc_o")
                    for j, dst in ((0, bc_e[:, 0, :]), (2, bc_e[:, 1, :]),
                                   (1, bc_o[:, 0, :]), (3, bc_o[:, 1, :])):
                        row = r_sb[0:1, j * 128:(j + 1) * 128]
                        nc.gpsimd.partition_broadcast(dst, row)
                    # normalize into paired-head layout: head 4g+j ->
                    # pair 2g + j//2, partition block (j%2)*64
                    pair = 2 * g
                    yv = yTn[:].rearrange("p (pr t) -> p pr t", t=TL)
                    ye = yv[0:64, pair:pair + 2, qb * 128:(qb + 1) * 128]
                    yo = yv[64:128, pair:pair + 2, qb * 128:(qb + 1) * 128]
                    yu4 = yu[0:64, :].rearrange("p (a b c) -> p a b c", b=2, c=128)
                    nc.vector.tensor_mul(ye, yu4[:, :, 0, :], bc_e[:])
                    nc.vector.tensor_mul(yo, yu4[:, :, 1, :], bc_o[:])

        # ======== phase 3: output projection ========
        with tc.tile_pool(name="ops", bufs=2, space="PSUM") as ops, \
             tc.tile_pool(name="osb", bufs=3) as osb:
            yv = yTn[:].rearrange("p (pr t) -> p pr t", t=TL)
            for tt in range(8):
                p0 = ops.tile([128, 512], F32, tag="po0")
                p1 = ops.tile([128, 512], F32, tag="po1")
                for half, pp in ((0, p0), (1, p1)):
                    for pr in range(8):
                        nc.tensor.matmul(
                            pp[:],
                            yv[:, pr, tt * 128:(tt + 1) * 128],
                            wp_sb[pr][:, half * 512:(half + 1) * 512],
                            start=(pr == 0), stop=(pr == 7))
                o_sb = osb.tile([128, C], F32, tag="o_sb")
                nc.vector.tensor_copy(o_sb[:, 0:512], p0[:])
                nc.vector.tensor_copy(o_sb[:, 512:1024], p1[:])
                nc.gpsimd.dma_start(out=out[tt * 128:(tt + 1) * 128, :],
                                    in_=o_sb[:])


_PROGRAM_CACHE = {}


def _get_program():
    if "nc" not in _PROGRAM_CACHE:
        _PROGRAM_CACHE["nc"] = _build_program()
    return _PROGRAM_CACHE["nc"]


def kernel(x, freqs_cos, freqs_sin, w_attn, b_attn, w_proj, b_proj):
    x = np.asarray(x, dtype=np.float32)
    freqs_cos = np.asarray(freqs_cos, dtype=np.float32)
    freqs_sin = np.asarray(freqs_sin, dtype=np.float32)
    w_attn = np.asarray(w_attn, dtype=np.float32)
    b_attn = np.asarray(b_attn, dtype=np.float32)
    w_proj = np.asarray(w_proj, dtype=np.float32)
    b_proj = np.asarray(b_proj, dtype=np.float32)
    assert not np.any(b_attn), "kernel assumes zero qkv bias"

    # q/k channel permutation: evens block then odds block, head-major
    qch = np.arange(H * HD).reshape(H, 32, 2)
    q_perm = np.concatenate([qch[:, :, 0].reshape(-1), qch[:, :, 1].reshape(-1)])
    kch = H * HD + np.arange(KV * HD).reshape(KV, 32, 2)
    k_perm = np.concatenate([kch[:, :, 0].reshape(-1), kch[:, :, 1].reshape(-1)])
    wqk = np.ascontiguousarray(
        w_attn[np.concatenate([q_perm, k_perm])].T).astype(BF)     # (1024, 1280)
    wv_h = np.ascontiguousarray(w_attn[(H + KV) * HD:].T).astype(BF)
    wp_h = np.ascontiguousarray(w_proj.T).astype(BF)

    cos4 = np.tile(freqs_cos.T, (4, 1)).astype(np.float32)    # (128, T)
    sin4 = np.tile(freqs_sin.T, (4, 1)).astype(np.float32)

    in_maps = []
    for core in range(8):
        b, h = divmod(core, 2)
        t0 = h * TL
        xs = np.zeros((TH, C), dtype=np.float32)
        lo = max(0, t0 - WIN)
        xs[TH - (t0 + TL - lo):] = x[b, lo:t0 + TL]
        vbv = np.zeros((1, 640), dtype=np.float32)
        if h == 0:
            vbv[0, :WIN] = NEG
        cpad = np.zeros((128, TH), dtype=np.float32)
        spad = np.zeros((128, TH), dtype=np.float32)
        cpad[:, TH - (t0 + TL - lo):] = cos4[:, lo:t0 + TL]
        spad[:, TH - (t0 + TL - lo):] = sin4[:, lo:t0 + TL]
        in_maps.append({
            "xT": np.ascontiguousarray(xs.T).astype(BF),
            "wqk": wqk, "wv": wv_h, "wp": wp_h,
            "cq": np.stack([cos4[:, t0:t0 + TL],
                            sin4[:, t0:t0 + TL]]).astype(BF),
            "ck": np.stack([cpad, spad]).astype(BF),
            "vb": vbv,
        })

    nc = _get_program()
    res = run_bass_kernel_spmd(nc, in_maps, list(range(8)))
    out = np.empty((B, T, C), dtype=np.float32)
    for core in range(8):
        b, h = divmod(core, 2)
        out[b, h * TL:(h + 1) * TL] = res.results[core]["out"]
    if np.any(b_proj):
        out += b_proj
    return out


# revision 6
# speedup vs baseline: 444.2726x; 444.2726x over previous
"""Sliding-window GQA causal self-attention block for 8 trn2 NeuronCores.

Sharding: batch (4) x T-halves (2) -> 8 cores, no collectives. Each core gets
x.T for its T-half plus a 256-row key/value halo and computes its (1024, 1024)
slice of the output.

Scores are computed keys-on-partitions (S.T) per (kv-group, q-block); exp on
ACT with scale=1/8 (softmax without max-subtraction -- scores are O(5));
band masks via gpsimd affine_select on the exp'd tile; att@v uses a ones
column appended to v so the softmax denominator falls out of the same matmul;
normalization via reciprocal + partition-broadcast DMA.
"""

import dataclasses

import numpy as np
import ml_dtypes

import concourse.bass as bass
import concourse.mybir as mybir
import concourse.tile as tile
from concourse import bacc
from concourse.bass_utils import run_bass_kernel_spmd

BF = ml_dtypes.bfloat16
F32 = mybir.dt.float32
BF16 = mybir.dt.bfloat16

B, T, C = 4, 2048, 1024
H, KV, HD = 16, 4, 64
WIN = 256
TL = T // 2            # 1024 own rows per core
TH = TL + WIN          # 1280 with halo
NEG = -30000.0


def _build_program():
    nc = bacc.Bacc("TRN2", target_bir_lowering=False, debug=False, num_devices=8)
    dt = mybir.dt
    xT = nc.dram_tensor("xT", [C, TH], dt.bfloat16, kind="ExternalInput").ap()
    wqk = nc.dram_tensor("wqk", [C, 1280], dt.bfloat16, kind="ExternalInput").ap()
    wv = nc.dram_tensor("wv", [C, 256], dt.bfloat16, kind="ExternalInput").ap()
    wp = nc.dram_tensor("wp", [C, C], dt.bfloat16, kind="ExternalInput").ap()
    cq = nc.dram_tensor("cq", [2, 128, TL], dt.bfloat16, kind="ExternalInput").ap()
    ck = nc.dram_tensor("ck", [2, 128, TH], dt.bfloat16, kind="ExternalInput").ap()
    vb = nc.dram_tensor("vb", [1, 640], dt.float32, kind="ExternalInput").ap()
    out = nc.dram_tensor("out", [TL, C], dt.float32, kind="ExternalOutput").ap()

    with tile.TileContext(nc) as tc:
        _kernel_body(tc, nc, xT, wqk, wv, wp, cq, ck, vb, out)
    nc.compile()
    return nc


def _kernel_body(tc, nc, xT, wqk, wv, wp, cq, ck, vb, out):
    import contextlib
    ctx = contextlib.ExitStack()
    with ctx:
        consts = ctx.enter_context(tc.tile_pool(name="consts", bufs=1))
        persist = ctx.enter_context(tc.tile_pool(name="persist", bufs=1))

        # ---- load persistent inputs ----
        xT_sb, wqk_sb, wv_sb, wp_sb = [], [], [], []
        for kc in range(8):
            t = persist.tile([128, TH], BF16, tag=f"xT{kc}")
            nc.gpsimd.dma_start(out=t[:], in_=xT[kc * 128:(kc + 1) * 128, :])
            xT_sb.append(t)
            t = persist.tile([128, 1280], BF16, tag=f"wqk{kc}")
            nc.gpsimd.dma_start(out=t[:], in_=wqk[kc * 128:(kc + 1) * 128, :])
            wqk_sb.append(t)
            t = persist.tile([128, 256], BF16, tag=f"wv{kc}")
            nc.gpsimd.dma_start(out=t[:], in_=wv[kc * 128:(kc + 1) * 128, :])
            wv_sb.append(t)
            t = persist.tile([128, C], BF16, tag=f"wp{kc}")
            nc.gpsimd.dma_start(out=t[:], in_=wp[kc * 128:(kc + 1) * 128, :])
            wp_sb.append(t)
        cq_sb = consts.tile([128, 2, TL], BF16)
        nc.gpsimd.dma_start(out=cq_sb[:, 0, :], in_=cq[0])
        nc.gpsimd.dma_start(out=cq_sb[:, 1, :], in_=cq[1])
        ck_sb = consts.tile([128, 2, TH], BF16)
        nc.gpsimd.dma_start(out=ck_sb[:, 0, :], in_=ck[0])
        nc.gpsimd.dma_start(out=ck_sb[:, 1, :], in_=ck[1])
        vb_sb = consts.tile([1, 640], F32)
        nc.gpsimd.dma_start(out=vb_sb[:], in_=vb)
        ones_sb = consts.tile([1, 512], F32)
        nc.vector.memset(ones_sb[:], 1.0)

        # persistent compute tensors
        qT = [persist.tile([64, TL], BF16, tag=f"qT{h}", name=f"qT{h}") for h in range(H)]
        kT = [persist.tile([64, TH], BF16, tag=f"kT{g}", name=f"kT{g}") for g in range(KV)]
        v65 = [persist.tile([128, 4 * 65], BF16, tag=f"v65_{i}", name=f"v65_{i}") for i in range(10)]
        yTn = persist.tile([128, 8 * TL], BF16, tag="yTn")  # paired heads x T

        # ======== phase 1: qkv projection + rope ========
        with tc.tile_pool(name="pps", bufs=1, space="PSUM") as pps, \
             tc.tile_pool(name="vps", bufs=2, space="PSUM") as vps, \
             tc.tile_pool(name="ropes", bufs=2) as ropes:

            def rope_pair(pe, po, cs_sb, tlen):
                e_sb = ropes.tile([128, tlen], BF16, tag="e_sb")
                o_sb = ropes.tile([128, tlen], BF16, tag="o_sb")
                nc.vector.tensor_copy(e_sb[:], pe[:, 0:tlen])
                nc.vector.tensor_copy(o_sb[:], po[:, 0:tlen])
                ne = ropes.tile([128, tlen], BF16, tag="r0")
                no_ = ropes.tile([128, tlen], BF16, tag="r1")
                t1 = ropes.tile([128, tlen], BF16, tag="r2")
                t2 = ropes.tile([128, tlen], BF16, tag="r3")
                nc.vector.tensor_mul(t1[:], e_sb[:], cs_sb[:, 0, 0:tlen])
                nc.vector.tensor_mul(t2[:], o_sb[:], cs_sb[:, 1, 0:tlen])
                nc.vector.tensor_sub(ne[:], t1[:], t2[:])
                nc.vector.tensor_mul(t1[:], e_sb[:], cs_sb[:, 1, 0:tlen])
                nc.vector.tensor_mul(t2[:], o_sb[:], cs_sb[:, 0, 0:tlen])
                nc.vector.tensor_add(no_[:], t1[:], t2[:])
                return ne, no_

            # q: wqk cols [0:512]=all-heads-evens, [512:1024]=all-heads-odds
            for c4 in range(4):
                pe = pps.tile([128, TH], F32, tag="pe")
                po = pps.tile([128, TH], F32, tag="po")
                for half in range(2):
                    for kc in range(8):
                        nc.tensor.matmul(
                            pe[:, half * 512:(half + 1) * 512],
                            wqk_sb[kc][:, c4 * 128:(c4 + 1) * 128],
                            xT_sb[kc][:, WIN + half * 512:WIN + (half + 1) * 512],
                            start=(kc == 0), stop=(kc == 7))
                    for kc in range(8):
                        nc.tensor.matmul(
                            po[:, half * 512:(half + 1) * 512],
                            wqk_sb[kc][:, 512 + c4 * 128:512 + (c4 + 1) * 128],
                            xT_sb[kc][:, WIN + half * 512:WIN + (half + 1) * 512],
                            start=(kc == 0), stop=(kc == 7))
                ne, no_ = rope_pair(pe, po, cq_sb, TL)
                for j in range(4):
                    h = c4 * 4 + j
                    nc.gpsimd.dma_start(out=qT[h][0:32, :],
                                        in_=ne[j * 32:(j + 1) * 32, :])
                    nc.gpsimd.dma_start(out=qT[h][32:64, :],
                                        in_=no_[j * 32:(j + 1) * 32, :])

            # k: wqk cols [1024:1152]=kv evens, [1152:1280]=kv odds, full TH
            pe = pps.tile([128, TH], F32, tag="pe")
            po = pps.tile([128, TH], F32, tag="po")
            for (n0, n1) in ((0, 512), (512, 1024), (1024, 1280)):
                for kc in range(8):
                    nc.tensor.matmul(pe[:, n0:n1], wqk_sb[kc][:, 1024:1152],
                                     xT_sb[kc][:, n0:n1],
                                     start=(kc == 0), stop=(kc == 7))
                for kc in range(8):
                    nc.tensor.matmul(po[:, n0:n1], wqk_sb[kc][:, 1152:1280],
                                     xT_sb[kc][:, n0:n1],
                                     start=(kc == 0), stop=(kc == 7))
            ne, no_ = rope_pair(pe, po, ck_sb, TH)
            for g in range(KV):
                nc.gpsimd.dma_start(out=kT[g][0:32, :],
                                    in_=ne[g * 32:(g + 1) * 32, :])
                nc.gpsimd.dma_start(out=kT[g][32:64, :],
                                    in_=no_[g * 32:(g + 1) * 32, :])

            # v: natural layout (t partitions, 4 heads x 64) + ones column
            for tcn in range(10):
                pv = vps.tile([128, 256], F32, tag="pv")
                for kc in range(8):
                    nc.tensor.matmul(pv[:], xT_sb[kc][:, tcn * 128:(tcn + 1) * 128],
                                     wv_sb[kc][:], start=(kc == 0), stop=(kc == 7))
                v3 = v65[tcn][:].rearrange("p (g c) -> p g c", c=65)
                nc.vector.tensor_copy(v3[:, :, 0:64],
                                      pv[:].rearrange("p (g c) -> p g c", c=64))
                nc.vector.memset(v3[:, :, 64:65], 1.0)

        # ======== phase 2: attention ========
        with tc.tile_pool(name="stps", bufs=2, space="PSUM") as stps, \
             tc.tile_pool(name="yups", bufs=2, space="PSUM") as yups, \
             tc.tile_pool(name="atts", bufs=3) as atts:
            for qb in range(8):
                for g in range(KV):
                    st = stps.tile([128, 4 * 384], F32, tag="st")
                    for j in range(4):          # 4 q-heads of this kv group
                        h = 4 * g + j
                        base = j * 384
                        for cc in range(3):     # key chunks qb, qb+1, qb+2
                            has_vb = (qb + cc) <= 1
                            nc.tensor.matmul(
                                st[:, base + cc * 128:base + (cc + 1) * 128],
                                kT[g][:, (qb + cc) * 128:(qb + cc + 1) * 128],
                                qT[h][:, qb * 128:(qb + 1) * 128],
                                start=True, stop=not has_vb)
                            if has_vb:
                                nc.tensor.matmul(
                                    st[:, base + cc * 128:base + (cc + 1) * 128],
                                    vb_sb[:, (qb + cc) * 128:(qb + cc + 1) * 128],
                                    ones_sb[:, 0:128],
                                    start=False, stop=True)
                    pt = atts.tile([128, 4 * 384], BF16, tag="pt")
                    nc.scalar.activation(pt[:], st[:],
                                         mybir.ActivationFunctionType.Exp,
                                         scale=0.125)
                    pt3 = pt[:].rearrange("p (h k) -> p h k", k=384)
                    # band mask chunk0: keep iff k_rel >= q_rel+1  (p - f - 1 >= 0)
                    nc.gpsimd.affine_select(
                        out=pt3[:, :, 0:128], in_=pt3[:, :, 0:128],
                        compare_op=mybir.AluOpType.is_ge, fill=0.0,
                        base=-1, channel_multiplier=1, pattern=[[0, 4], [-1, 128]])
                    # band mask chunk2: keep iff k_rel <= q_rel+256  (f - p >= 0)
                    nc.gpsimd.affine_select(
                        out=pt3[:, :, 256:384], in_=pt3[:, :, 256:384],
                        compare_op=mybir.AluOpType.is_ge, fill=0.0,
                        base=0, channel_multiplier=-1, pattern=[[0, 4], [1, 128]])
                    yu = yups.tile([65, 512], F32, tag="yu")
                    for j in range(4):
                        for cc in range(3):
                            nc.tensor.matmul(
                                yu[:, j * 128:(j + 1) * 128],
                                v65[qb + cc][:, g * 65:(g + 1) * 65],
                                pt3[:, j, cc * 128:(cc + 1) * 128],
                                start=(cc == 0), stop=(cc == 2))
                    # denominators -> reciprocal -> broadcast -> normalize
                    r_sb = atts.tile([1, 512], F32, tag="r_sb")
                    nc.vector.reciprocal(r_sb[:], yu[64:65, :])
                    bc_e = atts.tile([64, 2, 128], F32, tag="bc_e")
                    bc_o = atts.tile([64, 2, 128], F32, tag="bc_o")
                    for j, dst in ((0, bc_e[:, 0, :]), (2, bc_e[:, 1, :]),
                                   (1, bc_o[:, 0, :]), (3, bc_o[:, 1, :])):
                        row = r_sb[0:1, j * 128:(j + 1) * 128]
                        nc.gpsimd.partition_broadcast(dst, row)
                    # normalize into paired-head layout: head 4g+j ->
                    # pair 2g + j//2, partition block (j%2)*64
                    pair = 2 * g
                    yv = yTn[:].rearrange("p (pr t) -> p pr t", t=TL)
                    ye = yv[0:64, pair:pair + 2, qb * 128:(qb + 1) * 128]
                    yo = yv[64:128, pair:pair + 2, qb * 128:(qb + 1) * 128]
                    yu4 = yu[0:64, :].rearrange("p (a b c) -> p a b c", b=2, c=128)
                    nc.vector.tensor_mul(ye, yu4[:, :, 0, :], bc_e[:])
                    nc.vector.tensor_mul(yo, yu4[:, :, 1, :], bc_o[:])

        # ======== phase 3: output projection ========
        with tc.tile_pool(name="ops", bufs=2, space="PSUM") as ops, \
             tc.tile_pool(name="osb", bufs=3) as osb:
            yv = yTn[:].rearrange("p (pr t) -> p pr t", t=TL)
            for tt in range(8):
                p0 = ops.tile([128, 512], F32, tag="po0")
                p1 = ops.tile([128, 512], F32, tag="po1")
                for half, pp in ((0, p0), (1, p1)):
                    for pr in range(8):
                        nc.tensor.matmul(
                            pp[:],
                            yv[:, pr, tt * 128:(tt + 1) * 128],
                            wp_sb[pr][:, half * 512:(half + 1) * 512],
                            start=(pr == 0), stop=(pr == 7))
                o_sb = osb.tile([128, C], F32, tag="o_sb")
                nc.vector.tensor_copy(o_sb[:, 0:512], p0[:])
                nc.vector.tensor_copy(o_sb[:, 512:1024], p1[:])
                nc.gpsimd.dma_start(out=out[tt * 128:(tt + 1) * 128, :],
                                    in_=o_sb[:])


_PROGRAM_CACHE = {}


def _get_program():
    if "nc" not in _PROGRAM_CACHE:
        _PROGRAM_CACHE["nc"] = _build_program()
    return _PROGRAM_CACHE["nc"]


def prepare_in_maps(x, freqs_cos, freqs_sin, w_attn, b_attn, w_proj, b_proj):
    x = np.asarray(x, dtype=np.float32)
    freqs_cos = np.asarray(freqs_cos, dtype=np.float32)
    freqs_sin = np.asarray(freqs_sin, dtype=np.float32)
    w_attn = np.asarray(w_attn, dtype=np.float32)
    b_attn = np.asarray(b_attn, dtype=np.float32)
    w_proj = np.asarray(w_proj, dtype=np.float32)
    b_proj = np.asarray(b_proj, dtype=np.float32)
    assert not np.any(b_attn), "kernel assumes zero qkv bias"

    # q/k channel permutation: evens block then odds block, head-major
    qch = np.arange(H * HD).reshape(H, 32, 2)
    q_perm = np.concatenate([qch[:, :, 0].reshape(-1), qch[:, :, 1].reshape(-1)])
    kch = H * HD + np.arange(KV * HD).reshape(KV, 32, 2)
    k_perm = np.concatenate([kch[:, :, 0].reshape(-1), kch[:, :, 1].reshape(-1)])
    wqk = np.ascontiguousarray(
        w_attn[np.concatenate([q_perm, k_perm])].T).astype(BF)     # (1024, 1280)
    wv_h = np.ascontiguousarray(w_attn[(H + KV) * HD:].T).astype(BF)
    wp_h = np.ascontiguousarray(w_proj.T).astype(BF)

    cos4 = np.tile(freqs_cos.T, (4, 1)).astype(np.float32)    # (128, T)
    sin4 = np.tile(freqs_sin.T, (4, 1)).astype(np.float32)

    in_maps = []
    for core in range(8):
        b, h = divmod(core, 2)
        t0 = h * TL
        xs = np.zeros((TH, C), dtype=np.float32)
        lo = max(0, t0 - WIN)
        xs[TH - (t0 + TL - lo):] = x[b, lo:t0 + TL]
        vbv = np.zeros((1, 640), dtype=np.float32)
        if h == 0:
            vbv[0, :WIN] = NEG
        cpad = np.zeros((128, TH), dtype=np.float32)
        spad = np.zeros((128, TH), dtype=np.float32)
        cpad[:, TH - (t0 + TL - lo):] = cos4[:, lo:t0 + TL]
        spad[:, TH - (t0 + TL - lo):] = sin4[:, lo:t0 + TL]
        in_maps.append({
            "xT": np.ascontiguousarray(xs.T).astype(BF),
            "wqk": wqk, "wv": wv_h, "wp": wp_h,
            "cq": np.stack([cos4[:, t0:t0 + TL],
                            sin4[:, t0:t0 + TL]]).astype(BF),
            "ck": np.stack([cpad, spad]).astype(BF),
            "vb": vbv,
        })

    return in_maps


def kernel(**inputs):
    in_maps = prepare_in_maps(**inputs)
    nc = _get_program()
    res = run_bass_kernel_spmd(nc, in_maps, list(range(8)))
    return _gather(res, np.asarray(inputs["b_proj"], dtype=np.float32))


def _gather(res, b_proj):
    out = np.empty((B, T, C), dtype=np.float32)
    for core in range(8):
        b, h = divmod(core, 2)
        out[b, h * TL:(h + 1) * TL] = res.results[core]["out"]
    if np.any(b_proj):
        out += b_proj
    return out
